# revision 5
# baseline (speedup 1.0000x reference)
"""Trainium2 Bass kernel for nn_AttentionNet (GNN message passing, 2-layer GCN
with edge-MLP attention weights), 8 NeuronCores, tgt-sharded.

Strategy:
  - Core k owns target nodes [k*12500, (k+1)*12500) and ALL their in-edges.
  - Host: per-core node permutation (lexsort by per-chunk in-degree) ->
    node-major slot structure: per (tile t of 128 nodes, src-chunk c) a
    rectangular block of Dbar[t,c] slot-columns; real edges fill lanes,
    pads get a valid dummy index (0) and mask 0.
  - Device per core:
    stage A: edge MLP (ew = sigmoid(relu(exT@W1+b1)@W2+b2)) in wrap layout.
    deg = per-tile reduce of masked ew; dinv = 1/sqrt(deg+1).
    xs = dinv * (x @ Wc1) -> AllGather -> table1 [100352, 128] bf16.
    L1: per chunk, dma_gather rows, mult by ew, strided segment reduce -> acc1.
    h1s = dinv*relu(dinv*(acc1 + xs) + bc1) -> AllGather -> table2.
    L2: same gathers on table2 -> acc2.
    out = log_softmax(dinv*((acc2 + h1s) @ Wc2) + bc2).
"""
import sys
import numpy as np

sys.path.insert(0, "/opt/trn_rl_repo")

import ml_dtypes
import concourse.bass as bass
import concourse.tile as tile
import concourse.bacc as bacc
from concourse import mybir
from concourse.bass_utils import run_bass_kernel_spmd

NC = 8
N = 100000
NB = 12500
NBP = 12544           # padded per-core nodes (98 * 128)
NT = NBP // 128       # 98 node tiles
CH = 25088            # chunk size in global padded table (2 cores * 12544)
TBL = NC * NBP        # 100352
P = 128
EF, EFIL = 16, 32
NF, NFIL, CLS = 128, 64, 16
CALL_COLS = 64        # gather call width (<= 64 cols = 8192 idxs)

F32 = mybir.dt.float32
BF16 = mybir.dt.bfloat16
I16 = mybir.dt.int16
AF = mybir.ActivationFunctionType
OP = mybir.AluOpType


def _prep(x, edge_index, edge_x, W1, b1, W2, b2, Wc1, bc1, Wc2, bc2):
    src = np.asarray(edge_index[0]).astype(np.int64)
    tgt = np.asarray(edge_index[1]).astype(np.int64)
    x = np.asarray(x, np.float32)
    edge_x = np.asarray(edge_x, np.float32)

    core_of_tgt = tgt // NB
    chunk_of_src_node = np.minimum(src // (2 * NB), 3)  # chunk c = cores 2c,2c+1

    # --- per-core node perms (lexsort by per-chunk in-degree desc) ---
    perms, poss, degcs = [], [], []
    core_edges = []
    for k in range(NC):
        m = core_of_tgt == k
        e_ids = np.nonzero(m)[0]
        t_loc = tgt[e_ids] - k * NB
        ch = chunk_of_src_node[e_ids]
        d = np.zeros((NBP, 4), np.int64)
        np.add.at(d, (t_loc, ch), 1)
        perm = np.lexsort((-d[:, 3], -d[:, 2], -d[:, 1], -d[:, 0]))
        pos = np.empty(NBP, np.int64)
        pos[perm] = np.arange(NBP)
        perms.append(perm); poss.append(pos); degcs.append(d)
        core_edges.append((e_ids, t_loc, ch))

    # global table row for any src node
    src_core = src // NB
    # pos within its own core
    pos_all = np.empty(N, np.int64)
    for k in range(NC):
        ids = np.arange(k * NB, (k + 1) * NB)
        pos_all[ids] = poss[k][ids - k * NB]
    grow = src_core * NBP + pos_all[src]          # global row of src
    idx16_of_edge = grow - chunk_of_src_node * CH  # < 25088

    # --- per-(tile, chunk) Dbar, common across cores ---
    Dbar = np.zeros((NT, 4), np.int64)
    for k in range(NC):
        d_sorted = degcs[k][perms[k]].reshape(NT, P, 4)
        Dbar = np.maximum(Dbar, d_sorted.max(1))
    Wc_cols = Dbar.sum(0)                  # per-chunk total columns
    base_c = np.zeros(5, np.int64)
    base_c[1:] = np.cumsum(Wc_cols)
    SC_raw = int(base_c[4])
    SC = ((SC_raw + 11) // 12) * 12        # pad to x12 for stage-A batching
    EP = SC * P
    # per (c, t) column offset
    coloff = np.zeros((4, NT), np.int64)
    for c in range(4):
        coloff[c] = base_c[c] + np.concatenate(([0], np.cumsum(Dbar[:, c])[:-1]))

    # --- per-core slot arrays ---
    in_maps = []
    for k in range(NC):
        e_ids, t_loc, ch = core_edges[k]
        pos_t = poss[k][t_loc]             # slot lane/tile of target
        tt = pos_t // P
        pp = pos_t % P
        # rank within (node, chunk): sort by (ch, pos_t) then cumcount
        order = np.lexsort((pos_t, ch))
        ch_s, pos_s = ch[order], pos_t[order]
        key = ch_s * NBP + pos_s
        newgrp = np.concatenate(([True], key[1:] != key[:-1]))
        grp_start = np.maximum.accumulate(np.where(newgrp, np.arange(len(key)), 0))
        kk = np.arange(len(key)) - grp_start
        rank = np.empty(len(key), np.int64)
        rank[order] = kk
        col = coloff[ch, tt] + rank
        slot = col * P + pp

        exT = np.zeros((17, EP), ml_dtypes.bfloat16)
        exT[:16, slot] = edge_x[e_ids].T.astype(ml_dtypes.bfloat16)
        exT[16, slot] = 1.0
        unw = np.zeros(EP, np.int16)
        unw[slot] = idx16_of_edge[e_ids].astype(np.int16)
        w16 = unw.reshape(EP // 16, 16).T
        idxw = np.tile(w16, (8, 1)).copy()
        mask = np.zeros((P, SC), ml_dtypes.bfloat16)
        mask[pp, col] = 1.0
        xt = np.zeros((P, NBP), ml_dtypes.bfloat16)
        xrows = x[k * NB:(k + 1) * NB]     # [12500, 128]
        # node at pos i is perm[i]; cols are pos-ordered
        pvals = perms[k]
        valid = pvals < NB
        xt[:, valid.nonzero()[0]] = xrows[pvals[valid]].T.astype(ml_dtypes.bfloat16)
        in_maps.append({
            "exT": np.asarray(exT), "idxw": np.asarray(idxw),
            "maskw": np.asarray(mask), "xT": np.asarray(xt),
        })

    consts = dict(
        W1a=np.vstack([np.asarray(W1, np.float32),
                       np.asarray(b1, np.float32)[None, :]]).astype(ml_dtypes.bfloat16),
        W2r4=np.tile(np.asarray(W2, np.float32), (4, 1)).astype(ml_dtypes.bfloat16),
        Wc1=np.asarray(Wc1, np.float32).astype(ml_dtypes.bfloat16),
        Wc2=np.asarray(Wc2, np.float32).astype(ml_dtypes.bfloat16),
        bc1r=np.tile(np.asarray(bc1, np.float32)[None, :], (P, 1)),
        bc2r=np.tile(np.asarray(bc2, np.float32)[None, :], (P, 1)),
        ident=np.eye(P, dtype=ml_dtypes.bfloat16),
        b2f=float(np.asarray(b2, np.float32).reshape(-1)[0]),
    )
    meta = dict(Dbar=Dbar, coloff=coloff, base_c=base_c, SC=SC, EP=EP,
                perms=perms, poss=poss)
    return in_maps, consts, meta


def _build(consts, meta):
    Dbar, coloff, SC, EP = meta["Dbar"], meta["coloff"], meta["SC"], meta["EP"]
    nc = bacc.Bacc("TRN2", target_bir_lowering=False, debug=False, num_devices=NC)

    exT_d = nc.dram_tensor("exT", [17, EP], BF16, kind="ExternalInput")
    idxw_d = nc.dram_tensor("idxw", [P, EP // 16], I16, kind="ExternalInput")
    mask_d = nc.dram_tensor("maskw", [P, SC], BF16, kind="ExternalInput")
    xT_d = nc.dram_tensor("xT", [P, NBP], BF16, kind="ExternalInput")
    out_d = nc.dram_tensor("out", [NBP, CLS], F32, kind="ExternalOutput")

    W1a_d = nc.inline_tensor(consts["W1a"], "W1a")
    W2r4_d = nc.inline_tensor(consts["W2r4"], "W2r4")
    Wc1_d = nc.inline_tensor(consts["Wc1"], "Wc1")
    Wc2_d = nc.inline_tensor(consts["Wc2"], "Wc2")
    bc1r_d = nc.inline_tensor(consts["bc1r"], "bc1r")
    bc2r_d = nc.inline_tensor(consts["bc2r"], "bc2r")
    ident_d = nc.inline_tensor(np.asarray(consts["ident"]), "ident")
    b2f = consts["b2f"]

    with tile.TileContext(nc) as tc:
        with (
            tc.tile_pool(name="persist", bufs=1) as pers,
            tc.tile_pool(name="stream", bufs=2) as strm,
            tc.tile_pool(name="ps", bufs=2, space="PSUM") as psp,
            tc.tile_pool(name="ps1", bufs=1, space="PSUM") as psp1,
            tc.tile_pool(name="dram", bufs=1, space="DRAM") as drp,
        ):
            # ---- persistent tiles ----
            ew = pers.tile([P, SC], BF16)
            maskt = pers.tile([P, SC], BF16)
            xTt = pers.tile([P, NBP], BF16)
            acc1 = pers.tile([P, NT * NFIL], F32)
            acc2 = pers.tile([P, NT * NFIL], F32)
            xs_loc = pers.tile([P, NT * NFIL], BF16)
            h1s_loc = pers.tile([P, NT * NFIL], BF16)
            deg4 = pers.tile([P, 4 * NT], F32)
            dinv = pers.tile([P, NT], F32)
            scr = pers.tile([P, NT], F32)
            W1s = pers.tile([17, EFIL], BF16)
            W2s = pers.tile([P, 1], BF16)
            Wc1s = pers.tile([P, NFIL], BF16)
            Wc2s = pers.tile([NFIL, CLS], BF16)
            bc1s = pers.tile([P, NFIL], F32)
            bc2s = pers.tile([P, CLS], F32)
            idents = pers.tile([P, P], BF16)
            zeros = pers.tile([P, NT * NFIL], BF16)

            nc.sync.dma_start(maskt[:], mask_d[:])
            nc.sync.dma_start(xTt[:], xT_d[:])
            nc.sync.dma_start(W1s[:], W1a_d[:])
            nc.sync.dma_start(W2s[:], W2r4_d[:])
            nc.sync.dma_start(Wc1s[:], Wc1_d[:])
            nc.sync.dma_start(Wc2s[:], Wc2_d[:])
            nc.sync.dma_start(bc1s[:], bc1r_d[:])
            nc.sync.dma_start(bc2s[:], bc2r_d[:])
            nc.sync.dma_start(idents[:], ident_d[:])
            nc.vector.memset(zeros[:], 0)
            nc.vector.memset(acc1[:], 0)
            nc.vector.memset(acc2[:], 0)

            # ---- DRAM: bounce + tables ----
            bounce1 = drp.tile([TBL // NC, P], BF16)
            table1 = drp.tile([TBL, P], BF16)
            bounce2 = drp.tile([TBL // NC, P], BF16)
            table2 = drp.tile([TBL, P], BF16)

            # ---- stage A: edge MLP (3 groups of 512 per batch; psum base 0/32/64) ----
            nbatch = EP // 1536
            for b in range(nbatch):
                ext = strm.tile([17, 1536], BF16)
                nc.sync.dma_start(ext[:], exT_d[:, b * 1536:(b + 1) * 1536])
                h4p = psp.tile([96, 512], F32, space="PSUM")
                for g in range(3):
                    nc.tensor.matmul(
                        out=h4p[32 * g:32 * (g + 1), :],
                        lhsT=W1s[:], rhs=ext[:, 512 * g:512 * (g + 1)],
                        start=True, stop=True)
                h4s = strm.tile([96, 512], BF16)
                nc.scalar.activation(out=h4s[:], in_=h4p[:], func=AF.Relu)
                ewp = psp.tile([P, 12], F32, space="PSUM")
                for cl in range(12):
                    g, q = cl // 4, cl % 4
                    nc.tensor.matmul(
                        out=ewp[:, cl:cl + 1],
                        lhsT=h4s[32 * g:32 * (g + 1), 128 * q:128 * (q + 1)],
                        rhs=W2s[32 * g:32 * (g + 1), :],
                        start=True, stop=True)
                nc.scalar.activation(out=ew[:, b * 12:(b + 1) * 12], in_=ewp[:],
                                     func=AF.Sigmoid, bias=b2f)
            # mask pads
            nc.vector.tensor_tensor(out=ew[:], in0=ew[:], in1=maskt[:], op=OP.mult)

            # ---- deg / dinv ----
            for c in range(4):
                for t in range(NT):
                    D = int(Dbar[t, c])
                    if D == 0:
                        nc.vector.memset(deg4[:, c * NT + t:c * NT + t + 1], 0)
                        continue
                    o = int(coloff[c, t])
                    nc.vector.tensor_reduce(
                        out=deg4[:, c * NT + t:c * NT + t + 1],
                        in_=ew[:, o:o + D], axis=mybir.AxisListType.X, op=OP.add)
            nc.vector.tensor_reduce(
                out=dinv[:],
                in_=deg4[:].rearrange("p (c t) -> p c t", c=4).transpose([0, 2, 1]),
                axis=mybir.AxisListType.X, op=OP.add)
            # dinv = 1/sqrt(deg+1)
            nc.scalar.activation(out=scr[:], in_=dinv[:], func=AF.Sqrt, bias=1.0)
            nc.vector.reciprocal(out=dinv[:], in_=scr[:])

            # ---- xs = dinv * (x @ Wc1), write bounce1 ----
            for t in range(NT):
                xp = psp1.tile([P, NFIL], F32, space="PSUM")
                nc.tensor.matmul(out=xp[:], lhsT=xTt[:, t * P:(t + 1) * P],
                                 rhs=Wc1s[:], start=True, stop=True)
                nc.scalar.activation(out=xs_loc[:, t * NFIL:(t + 1) * NFIL],
                                     in_=xp[:], func=AF.Copy,
                                     scale=dinv[:, t:t + 1])
            nc.sync.dma_start(
                bounce1[:, :NFIL].rearrange("(t p) f -> p t f", p=P),
                xs_loc[:].rearrange("p (t f) -> p t f", f=NFIL))
            # zero the pad cols 64:128 once
            nc.sync.dma_start(
                bounce1[:, NFIL:].rearrange("(t p) f -> p t f", p=P),
                zeros[:].rearrange("p (t f) -> p t f", f=NFIL))
            nc.gpsimd.collective_compute(
                "AllGather", OP.bypass, replica_groups=[list(range(NC))],
                ins=[bounce1[:].opt()], outs=[table1[:].opt()])

            # ---- gather+reduce loop (shared for L1/L2) ----
            def layer_loop(table, acc):
                for c in range(4):
                    a = int(coloff[c, 0])
                    end_c = int(coloff[c, NT - 1] + Dbar[NT - 1, c])
                    o = a
                    while o < end_c:
                        w = min(CALL_COLS, end_c - o)
                        ni = w * P
                        idxt = strm.tile([P, w * 8], I16)
                        nc.sync.dma_start(idxt[:], idxw_d[:, o * 8:(o + w) * 8])
                        msgs = strm.tile([P, w, P], BF16)
                        nc.gpsimd.dma_gather(
                            out_ap=msgs[:], in_ap=table[c * CH:(c + 1) * CH, :],
                            idxs_ap=idxt[:], num_idxs=ni, num_idxs_reg=ni,
                            elem_size=P, single_packet=False)
                        scl = strm.tile([P, w, NFIL], BF16)
                        nc.vector.tensor_tensor(
                            out=scl[:],
                            in0=msgs[:, :, :NFIL],
                            in1=ew[:, o:o + w].unsqueeze(2).to_broadcast(
                                [P, w, NFIL]),
                            op=OP.mult)
                        # per-tile blocks inside [o, o+w)
                        for t in range(NT):
                            bs = int(coloff[c, t]); be = bs + int(Dbar[t, c])
                            lo, hi = max(bs, o), min(be, o + w)
                            if lo >= hi:
                                continue
                            D = hi - lo
                            tmp = strm.tile([P, NFIL], F32)
                            nc.vector.tensor_reduce(
                                out=tmp[:],
                                in_=scl[:, lo - o:hi - o, :].transpose([0, 2, 1]),
                                axis=mybir.AxisListType.X, op=OP.add)
                            nc.vector.tensor_tensor(
                                out=acc[:, t * NFIL:(t + 1) * NFIL],
                                in0=acc[:, t * NFIL:(t + 1) * NFIL],
                                in1=tmp[:], op=OP.add)
                        o += w

            layer_loop(table1, acc1)

            # ---- h1s ----
            for t in range(NT):
                sl = slice(t * NFIL, (t + 1) * NFIL)
                t1 = strm.tile([P, NFIL], F32)
                nc.vector.tensor_tensor(out=t1[:], in0=acc1[:, sl],
                                        in1=xs_loc[:, sl], op=OP.add)
                t2 = strm.tile([P, NFIL], F32)
                nc.scalar.activation(out=t2[:], in_=t1[:], func=AF.Copy,
                                     scale=dinv[:, t:t + 1])
                nc.vector.tensor_tensor(out=t2[:], in0=t2[:], in1=bc1s[:],
                                        op=OP.add)
                nc.vector.tensor_scalar_max(t2[:], t2[:], 0.0)
                nc.scalar.activation(out=h1s_loc[:, sl], in_=t2[:], func=AF.Copy,
                                     scale=dinv[:, t:t + 1])
            nc.sync.dma_start(
                bounce2[:, :NFIL].rearrange("(t p) f -> p t f", p=P),
                h1s_loc[:].rearrange("p (t f) -> p t f", f=NFIL))
            nc.sync.dma_start(
                bounce2[:, NFIL:].rearrange("(t p) f -> p t f", p=P),
                zeros[:].rearrange("p (t f) -> p t f", f=NFIL))
            nc.gpsimd.collective_compute(
                "AllGather", OP.bypass, replica_groups=[list(range(NC))],
                ins=[bounce2[:].opt()], outs=[table2[:].opt()])

            layer_loop(table2, acc2)

            # ---- final: out = log_softmax(dinv*((acc2+h1s)@Wc2)+bc2) ----
            for t in range(NT):
                sl = slice(t * NFIL, (t + 1) * NFIL)
                u = strm.tile([P, NFIL], BF16)
                nc.vector.tensor_tensor(out=u[:], in0=acc2[:, sl],
                                        in1=h1s_loc[:, sl], op=OP.add)
                utp = psp1.tile([NFIL, P], BF16, space="PSUM")
                nc.tensor.transpose(out=utp[:], in_=u[:], identity=idents[:])
                uts = strm.tile([NFIL, P], BF16)
                nc.vector.tensor_copy(out=uts[:], in_=utp[:])
                vp = psp1.tile([P, CLS], F32, space="PSUM")
                nc.tensor.matmul(out=vp[:], lhsT=uts[:], rhs=Wc2s[:],
                                 start=True, stop=True)
                z = strm.tile([P, CLS], F32)
                nc.scalar.activation(out=z[:], in_=vp[:], func=AF.Copy,
                                     scale=dinv[:, t:t + 1])
                nc.vector.tensor_tensor(out=z[:], in0=z[:], in1=bc2s[:],
                                        op=OP.add)
                nmx = strm.tile([P, 1], F32)
                nc.vector.tensor_reduce(out=nmx[:], in_=z[:],
                                        axis=mybir.AxisListType.X, op=OP.max,
                                        negate=True)
                et = strm.tile([P, CLS], F32)
                sume = strm.tile([P, 1], F32)
                nc.scalar.activation(out=et[:], in_=z[:], func=AF.Exp,
                                     bias=nmx[:], accum_out=sume[:])
                lse = strm.tile([P, 1], F32)
                nc.scalar.activation(out=lse[:], in_=sume[:], func=AF.Ln)
                res = strm.tile([P, CLS], F32)
                nc.vector.tensor_scalar(out=res[:], in0=z[:], scalar1=nmx[:],
                                        scalar2=lse[:], op0=OP.add,
                                        op1=OP.subtract)
                nc.sync.dma_start(out_d[t * P:(t + 1) * P, :], res[:])

    nc.compile()
    return nc


_last = {}


def kernel(**inputs):
    in_maps, consts, meta = _prep(**inputs)
    nc = _build(consts, meta)
    _last.update(nc=nc, in_maps=in_maps, meta=meta)
    res = run_bass_kernel_spmd(nc, in_maps, core_ids=list(range(NC)))
    _last["exec_time_ns"] = getattr(res, "exec_time_ns", None)
    out = np.zeros((N, CLS), np.float32)
    for k in range(NC):
        ok = res.results[k]["out"]          # [NBP, CLS] pos-ordered
        perm = meta["perms"][k]
        valid = perm < NB
        out[k * NB + perm[valid]] = ok[valid.nonzero()[0]]
    return out
